# revision 1
# baseline (speedup 1.0000x reference)
"""DbrxAttention (B=1, S=2048, D=6144, 48 q heads / 8 kv heads, rope, causal)
on 8 Trainium2 NeuronCores.

Sharding: tensor-parallel across heads. Core c owns q heads [6c, 6c+6) and kv
head c (GQA groups align: q head h uses kv head h//6, so each core's 6 q heads
share exactly its 1 kv head). Wqkv output dim and Wout input dim are sharded;
a ReduceScatter after out_proj sums the partial outputs, and the host
concatenates the 8 row-shards.

Dataflow is fully "transposed" ([feature, seq] layouts) so every matmul
contracts over the partition axis with no on-device layout changes except
f16 xbar DMA-transposes for V and the softmax probabilities.

Numerics: f16 matmul operands, fp32 PSUM accumulation, fp32 softmax with a
constant shift (exp(s - 12); scores for this problem are < 16 so no row-max
pass is needed), fp32 normalize before the f16 cast so tiny rows don't flush
to zero.
"""

import numpy as np

N_CORES = 8
S = 2048
D = 6144
HD = 128
NQH = 6                 # q heads per core
P = 128
QKV_ROWS = 1024         # per-core Wqkv rows: 768 q + 128 k + 128 v
NKT = S // P            # 16 key tiles
NQC = 4                 # q chunks
QCW = S // NQC          # 512
DT = D // P             # 48 d-model tiles
SCALE = HD ** -0.5
CAP = 12.0              # softmax constant shift
CLIP = 8.0

_cached_nc = None


def _build_nc():
    import concourse.mybir as mybir
    import concourse.tile as tile
    from concourse import bacc

    f16, f32 = mybir.dt.float16, mybir.dt.float32
    add_op = mybir.AluOpType.add
    mult_op = mybir.AluOpType.mult
    min_op = mybir.AluOpType.min
    max_op = mybir.AluOpType.max
    X = mybir.AxisListType.X
    Exp = mybir.ActivationFunctionType.Exp

    nc = bacc.Bacc("TRN2", target_bir_lowering=False, debug=False,
                   num_devices=N_CORES)

    hiddenT = nc.dram_tensor("hiddenT", [D, S], f16, kind="ExternalInput").ap()
    wqkvT = nc.dram_tensor("wqkvT", [D, QKV_ROWS], f16, kind="ExternalInput").ap()
    woutT = nc.dram_tensor("woutT", [NQH * HD, D], f16, kind="ExternalInput").ap()
    ccq = nc.dram_tensor("ccq", [P, S], f16, kind="ExternalInput").ap()
    ssq = nc.dram_tensor("ssq", [P, S], f16, kind="ExternalInput").ap()
    cck = nc.dram_tensor("cck", [P, S], f16, kind="ExternalInput").ap()
    ssk = nc.dram_tensor("ssk", [P, S], f16, kind="ExternalInput").ap()
    ident = nc.dram_tensor("ident", [P, P], f16, kind="ExternalInput").ap()
    maskd = nc.dram_tensor("maskd", [P, P], f16, kind="ExternalInput").ap()
    outs = [nc.dram_tensor(f"out{g}", [D // N_CORES, QCW], f16,
                           kind="ExternalOutput").ap() for g in range(NQC)]

    with tile.TileContext(nc) as tc:
        with (
            tc.tile_pool(name="const", bufs=1) as const,
            tc.tile_pool(name="wout", bufs=1) as woutp,
            tc.tile_pool(name="kv", bufs=1) as kvp,
            tc.tile_pool(name="stream", bufs=1) as stream,
            tc.tile_pool(name="work", bufs=1) as work,
            tc.tile_pool(name="stats", bufs=1) as stats,
            tc.tile_pool(name="dram", bufs=1, space="DRAM") as dram,
        ):
            ident_sb = const.tile([P, P], f16, tag="ident")
            nc.sync.dma_start(ident_sb[:], ident[:])
            maskd_sb = const.tile([P, P], f16, tag="maskd")
            nc.sync.dma_start(maskd_sb[:], maskd[:])
            negcap = const.tile([P, 1], f32, tag="negcap")
            nc.vector.memset(negcap[:], -CAP)

            wout_sb = const.tile([P, NQH, D], f16, tag="wout")
            for h in range(NQH):
                nc.gpsimd.dma_start(
                    wout_sb[:, h, :], woutT[P * h:P * (h + 1), :])

            k_sb = kvp.tile([P, S], f16, tag="k_sb")
            v_sb = kvp.tile([P, NKT, P], f16, tag="v_sb")
            pending_rs = None

            for qc in range(NQC):
                cs = slice(QCW * qc, QCW * (qc + 1))

                # ---- stage 1: qkv projection + clip + rope for this q chunk
                tabs = {}
                for nm, src in (("ccq", ccq), ("ssq", ssq),
                                ("cck", cck), ("ssk", ssk)):
                    t = stream.tile([P, QCW], f16, tag=nm, bufs=2, name=nm)
                    nc.sync.dma_start(t[:], src[:, cs])
                    tabs[nm] = t
                q_qc = work.tile([P, NQH, QCW], f16, tag="q_qc", bufs=2,
                                 name="q_qc")

                with tc.tile_pool(name=f"ps1_{qc}", bufs=1,
                                  space="PSUM") as ps1p:
                    ps1 = [ps1p.tile([P, QCW], f32, tag=f"mm{m}",
                                     name=f"ps1_{m}") for m in range(8)]
                    for kt in range(DT):
                        h_t = stream.tile([P, QCW], f16, tag="ht", bufs=6,
                                          name="h_t")
                        nc.gpsimd.dma_start(
                            h_t[:], hiddenT[P * kt:P * (kt + 1), cs])
                        w_t = stream.tile([P, QKV_ROWS], f16, tag="wt",
                                          bufs=8, name="w_t")
                        nc.gpsimd.dma_start(
                            w_t[:], wqkvT[P * kt:P * (kt + 1), :])
                        for m in (6, 7, 0, 1, 2, 3, 4, 5):
                            last_s1_mm = nc.tensor.matmul(
                                ps1[m][:], w_t[:, P * m:P * (m + 1)], h_t[:],
                                start=(kt == 0), stop=(kt == DT - 1))

                    for m in (6, 0, 1, 7, 2, 3, 4, 5):
                        if m == 7:  # v: clip -> f16, then xbar-transpose
                            vT = work.tile([P, QCW], f16, tag="vT", bufs=2,
                                           name="vT")
                            nc.vector.tensor_scalar(
                                vT[:], ps1[7][:], CLIP, -CLIP, min_op, max_op)
                            nc.sync.dma_start_transpose(
                                v_sb[:, 4 * qc:4 * (qc + 1), :], vT[:])
                            continue
                        a_t = work.tile([P, QCW], f32, tag="ropeA", bufs=2,
                                        name="a_t")
                        nc.vector.tensor_scalar(
                            a_t[:], ps1[m][:], CLIP, -CLIP, min_op, max_op)
                        b_t = work.tile([P, QCW], f32, tag="ropeB", bufs=2,
                                        name="b_t")
                        nc.gpsimd.dma_start(b_t[0:64, :], a_t[64:128, :])
                        nc.gpsimd.dma_start(b_t[64:128, :], a_t[0:64, :])
                        cc_t = tabs["cck"] if m == 6 else tabs["ccq"]
                        ss_t = tabs["ssk"] if m == 6 else tabs["ssq"]
                        e_t = work.tile([P, QCW], f32, tag="ropeE", bufs=2,
                                        name="e_t")
                        nc.vector.tensor_tensor(
                            e_t[:], a_t[:], cc_t[:], mult_op)
                        f_t = work.tile([P, QCW], f32, tag="ropeF", bufs=2,
                                        name="f_t")
                        nc.vector.tensor_tensor(
                            f_t[:], b_t[:], ss_t[:], mult_op)
                        dst = k_sb[:, cs] if m == 6 else q_qc[:, m, :]
                        nc.vector.tensor_tensor(dst, e_t[:], f_t[:], add_op)

                # ---- stage 2: attention for this q chunk
                attnT = work.tile([P, NQH, QCW], f16, tag="attnT", bufs=2,
                                  name="attnT")
                with tc.tile_pool(name=f"ps2_{qc}", bufs=1,
                                  space="PSUM") as ps2p:
                    njt = 4 * (qc + 1)  # key tiles active for this chunk
                    for h in range(NQH):
                        probsT = work.tile([P, NKT, QCW], f16, tag="probsT",
                                           bufs=2, name="probsT")
                        for jl in range(1, 4):
                            nc.vector.memset(
                                probsT[:, 4 * qc + jl, :P * jl], 0.0)
                        for il in range(4):
                            i = 4 * qc + il
                            L = P * (i + 1)
                            nkc = (L + 511) // 512
                            s_all = stats.tile([P, 4], f32, tag="s_all",
                                               bufs=3, name="s_all")
                            probs16 = work.tile([P, S], f16, tag="probs16",
                                                bufs=2, name="probs16")
                            pscs = []
                            for kc in range(nkc):
                                n = min(512, L - 512 * kc)
                                last = kc == nkc - 1
                                psc = ps2p.tile([P, 512], f32, tag="sc",
                                                bufs=6, name="psc")
                                nc.tensor.matmul(
                                    psc[:, :n],
                                    q_qc[:, h, P * il:P * (il + 1)],
                                    k_sb[:, 512 * kc:512 * kc + n],
                                    start=True, stop=not last)
                                if last:
                                    # causal mask: accumulate -60000 into the
                                    # upper triangle of the diagonal block
                                    nc.tensor.matmul(
                                        psc[:, n - P:n], ident_sb[:],
                                        maskd_sb[:], start=False, stop=True)
                                pscs.append((psc, n, kc))
                            p32s = []
                            for psc, n, kc in pscs:
                                p32 = work.tile([P, 512], f32, tag="p32",
                                                bufs=5, name="p32")
                                nc.scalar.activation(
                                    p32[:, :n],
                                    psc[:, :n], Exp, bias=negcap[:],
                                    scale=1.0,
                                    accum_out=s_all[:, kc:kc + 1])
                                p32s.append((p32, n, kc))
                            ssum = stats.tile([P, 1], f32, tag="ssum",
                                              bufs=3, name="ssum")
                            nc.vector.reduce_sum(ssum[:], s_all[:, :nkc],
                                                 axis=X)
                            rcp = stats.tile([P, 1], f32, tag="rcp",
                                             bufs=3, name="rcp")
                            nc.vector.reciprocal(rcp[:], ssum[:])
                            for p32, n, kc in p32s:
                                nc.vector.tensor_scalar_mul(
                                    probs16[:, 512 * kc:512 * kc + n],
                                    p32[:, :n], rcp[:])
                            nc.sync.dma_start_transpose(
                                probsT[:, :i + 1, P * il:P * (il + 1)],
                                probs16[:, :L])
                        ps_pv = ps2p.tile([P, 512], f32, tag="pv", bufs=2,
                                          name="ps_pv")
                        for j in range(njt):
                            last_pv_mm = nc.tensor.matmul(
                                ps_pv[:], v_sb[:, j, :], probsT[:, j, :],
                                start=(j == 0), stop=(j == njt - 1))
                        nc.vector.tensor_copy(attnT[:, h, :], ps_pv[:])

                if pending_rs is not None:
                    pqc, p_outT, p_rsout = pending_rs
                    cc_inst = nc.gpsimd.collective_compute(
                        "ReduceScatter",
                        mybir.AluOpType.add,
                        replica_groups=[list(range(N_CORES))],
                        ins=[p_outT[:]],
                        outs=[p_rsout[:]],
                    )
                    tile.add_dep_helper(
                        cc_inst.ins, last_pv_mm.ins, sync=True,
                        reason="overlap RS with outproj")
                    nc.sync.dma_start(outs[pqc][:], p_rsout[:])
                    pending_rs = None

                # ---- stage 3: output projection + ReduceScatter
                outT_qc = dram.tile([D, QCW], f16, tag=f"outT{qc}",
                                    name=f"outT{qc}")
                rs_out = dram.tile([D // N_CORES, QCW], f16,
                                   tag=f"rsout{qc}", name=f"rsout{qc}")
                with tc.tile_pool(name=f"ps3_{qc}", bufs=1,
                                  space="PSUM") as ps3p:
                    for dm in range(DT):
                        pso = ps3p.tile([P, QCW], f32, tag="po", bufs=4,
                                        name="pso")
                        for h6 in range(NQH):
                            nc.tensor.matmul(
                                pso[:],
                                wout_sb[:, h6, P * dm:P * (dm + 1)],
                                attnT[:, h6, :],
                                start=(h6 == 0), stop=(h6 == NQH - 1))
                        ot = work.tile([P, QCW], f16, tag="ot", bufs=3,
                                       name="ot")
                        nc.scalar.copy(ot[:], pso[:])
                        nc.gpsimd.dma_start(
                            outT_qc[P * dm:P * (dm + 1), :], ot[:])
                pending_rs = (qc, outT_qc, rs_out)

            if pending_rs is not None:
                pqc, p_outT, p_rsout = pending_rs
                nc.gpsimd.collective_compute(
                    "ReduceScatter",
                    mybir.AluOpType.add,
                    replica_groups=[list(range(N_CORES))],
                    ins=[p_outT[:]],
                    outs=[p_rsout[:]],
                )
                nc.sync.dma_start(outs[pqc][:], p_rsout[:])

    nc.compile()
    return nc


def _get_nc():
    global _cached_nc
    if _cached_nc is None:
        _cached_nc = _build_nc()
    return _cached_nc


def kernel(**inputs):
    from concourse.bass_utils import run_bass_kernel_spmd

    hs = np.asarray(inputs["hidden_states"])[0].astype(np.float32)   # [S, D]
    Wqkv = np.asarray(inputs["Wqkv"]).astype(np.float32)             # [8192, D]
    Wout = np.asarray(inputs["Wout"]).astype(np.float32)             # [D, D]
    pos = np.asarray(inputs["position_ids"])[0]

    f16 = np.float16
    hiddenT = np.ascontiguousarray(hs.T).astype(f16)                 # [D, S]
    WT = Wqkv.T.astype(f16)                                          # [D, 8192]
    WoT = Wout.T.astype(f16)                                         # [D, D]

    half = HD // 2
    inv = (1.0 / (500000.0 ** (np.arange(half, dtype=np.float32) * 2.0 / HD)))
    ang = pos.astype(np.float32)[:, None] * inv[None, :].astype(np.float32)
    cos = np.cos(ang).T.astype(np.float32)                           # [64, S]
    sin = np.sin(ang).T.astype(np.float32)
    cc = np.concatenate([cos, cos], axis=0)                          # [128, S]
    ss = np.concatenate([-sin, sin], axis=0)
    ccq = np.ascontiguousarray((cc * SCALE).astype(f16))
    ssq = np.ascontiguousarray((ss * SCALE).astype(f16))
    cck = np.ascontiguousarray(cc.astype(f16))
    ssk = np.ascontiguousarray(ss.astype(f16))
    idx = np.arange(P)
    identm = np.eye(P, dtype=np.float16)
    maskdm = np.where(idx[None, :] > idx[:, None], -60000.0, 0.0).astype(np.float16)

    in_maps = []
    for c in range(N_CORES):
        wq = np.ascontiguousarray(np.concatenate([
            WT[:, 768 * c:768 * (c + 1)],
            WT[:, D + P * c:D + P * (c + 1)],
            WT[:, D + 1024 + P * c:D + 1024 + P * (c + 1)],
        ], axis=1))
        wo = np.ascontiguousarray(WoT[768 * c:768 * (c + 1), :])
        in_maps.append(dict(hiddenT=hiddenT, wqkvT=wq, woutT=wo,
                            ccq=ccq, ssq=ssq, cck=cck, ssk=ssk,
                            ident=identm, maskd=maskdm))

    nc = _get_nc()
    res = run_bass_kernel_spmd(nc, in_maps, core_ids=list(range(N_CORES)))
    kernel._last_results = res

    outT = np.empty((D, S), np.float32)
    for qc in range(NQC):
        for c in range(N_CORES):
            outT[768 * c:768 * (c + 1), QCW * qc:QCW * (qc + 1)] = \
                res.results[c][f"out{qc}"].astype(np.float32)
    return np.ascontiguousarray(outT.T)[None]

